# revision 41
# baseline (speedup 1.0000x reference)
"""Trainium2 Bass kernel for nn_Loss_89730456748593 (MMCE + cross-entropy).

Math (see reference): for each of S=8 MC samples over a [B=2048, C=20] logit
matrix:
  p_i   = max softmax prob of row i
  acc_i = (argmax_i == label_i)
  w_i   = (acc_i - p_i) * (acc_i ? 1/B : 1/(ncorrect-B))
  MMCE_s = sqrt( (1/B^2) * sum_ij exp(-|p_i-p_j|/0.4) w_i w_j )
  loss = 2*mean_s(MMCE_s) + mean cross-entropy over all S*B rows

Sharding: data-parallel over S — core s computes sample s's MMCE and partial
CE sum; the host averages the 8 per-core scalar pairs (the "all-reduce mean").

Device algorithm per core (histogram formulation):
  - quantize q_i = round(p_i * 127). The Laplacian kernel then only depends
    on the bin pair: K = T[q_i, q_j], T[a,b] = exp(-2.5*|a-b|/127) — a
    128x128 compile-time constant (NEFF-embedded).  sum_ij K w_i w_j ==
    h^T T h with the signed histogram h[a] = sum_{i: q_i=a} w_i.  Bin width
    1/127 puts ~<=2% worst-case on K and ~2e-5 relative on the final loss
    (the MMCE term is 0.006% of the loss; cross-entropy, which dominates,
    is computed exactly).
  - w is split as w = w_corr + rin * w_inc with w_corr = (acc-p)*acc/B and
    w_inc = (acc-p)*(1-acc), both independent of ncorrect, so the histogram
    matmuls (lhsT = [w_corr | w_inc], m=2) overlap the GpSimd all-reduce
    that produces rin; rin folds in linearly afterwards.
  - histogram: one-hot oh[i, a] = (q_i == a) via 16 single-src bf16
    tensor_scalar compares (4x DVE mode), then 16 accumulating PE matmuls
    contract over the 128 partitions into PSUM [2, 128].
  - h^T T h: gather h onto partitions via an SBUF->SBUF DMA, one matmul
    against T gives Th, a dot + partition matmul give the total;
    MMCE = exp(0.5*ln(total) + ln(1/B)) (stays in the natural_log_exp ACT
    table set — no sqrt table load).
"""

import math

import numpy as np

import concourse.bacc as bacc
import concourse.bass_isa as bass_isa
import concourse.tile as tile
from concourse import hw_specs, mybir
from concourse.bass_utils import run_bass_kernel_spmd

AF = mybir.ActivationFunctionType
OP = mybir.AluOpType
AX = mybir.AxisListType
F32 = mybir.dt.float32
BF16 = mybir.dt.bfloat16
I32 = mybir.dt.int32

S, B, C = 8, 2048, 20
P = 128
NB = B // P  # 16 rows per partition
NBINS = 128
QSCALE = float(NBINS - 1)  # p in [0,1] -> bins 0..127
INV_BW = 2.5  # 1 / 0.4
LN_INV_B = math.log(1.0 / B)
N_CORES = 8

# Pin the ACT table set: every activation this kernel uses (Exp, Ln, Copy,
# Identity) lives in "natural_log_exp_and_others". Left to its own devices
# the table chooser bounces between the exp-only and ln-only sets on every
# Exp<->Ln transition (1.28us per table load). Emptying every other set
# (order preserved, so act_func_set_id stays a valid index into
# act_info.json) forces the combined set -> 1 load.
_orig_get_activation_tables = hw_specs.get_activation_tables.__wrapped__


def _pinned_activation_tables(module_arch):
    tables = _orig_get_activation_tables(module_arch)
    keep = "natural_log_exp_and_others"
    need = {AF.Exp, AF.Ln, AF.Copy, AF.Identity}
    if keep in tables and need <= tables[keep]:
        tables = {k: (v if k == keep else set()) for k, v in tables.items()}
    return tables


_pinned_cache = {}


def _pinned_cached(module_arch):
    if module_arch not in _pinned_cache:
        _pinned_cache[module_arch] = _pinned_activation_tables(module_arch)
    return _pinned_cache[module_arch]


hw_specs.get_activation_tables = _pinned_cached
bacc.get_activation_tables = _pinned_cached


def _kernel_table():
    """T[a,b] = exp(-2.5|a-b|/127) as a single [128,128] bf16 chunk
    (symmetric, so it is its own lhsT)."""
    import ml_dtypes

    a = np.arange(NBINS, dtype=np.float64)
    t = np.exp(-INV_BW / QSCALE * np.abs(a[:, None] - a[None, :]))
    return np.ascontiguousarray(t).astype(ml_dtypes.bfloat16)


def _build_body(nc, tc, logits, labels, out, t_dram):
    consts = tc.alloc_tile_pool(name="consts", bufs=1)
    keep = tc.alloc_tile_pool(name="keep", bufs=1)
    work = tc.alloc_tile_pool(name="work", bufs=2)
    ps_misc = tc.alloc_tile_pool(name="ps_misc", bufs=2, space="PSUM")
    pools = [consts, keep, work, ps_misc]

    # ---- constants ----
    iota_c = consts.tile([P, C], F32)
    nc.gpsimd.iota(
        iota_c, pattern=[[1, C]], base=0, channel_multiplier=0,
        allow_small_or_imprecise_dtypes=True,
    )
    iota_b = consts.tile([P, NBINS], BF16)  # 0..255: exact in bf16
    nc.gpsimd.iota(
        iota_b, pattern=[[1, NBINS]], base=0, channel_multiplier=0,
        allow_small_or_imprecise_dtypes=True,
    )
    ones_k128 = consts.tile([P, 1], BF16)
    nc.vector.memset(ones_k128, 1.0)
    lninvb = consts.tile([1, 1], F32)
    nc.vector.memset(lninvb, LN_INV_B)
    tsb = consts.tile([P, NBINS], BF16)
    nc.sync.dma_start(out=tsb, in_=t_dram)

    # ---- load inputs ----
    lg = keep.tile([P, NB, C], F32)
    nc.sync.dma_start(out=lg, in_=logits.rearrange("(p n) c -> p n c", p=P))
    lab_i = work.tile([P, NB], I32)
    nc.sync.dma_start(out=lab_i, in_=labels.rearrange("(p n) -> p n", p=P))

    # ---- per-row stats ----
    labf = keep.tile([P, NB], F32)
    nc.vector.tensor_copy(out=labf, in_=lab_i)  # int32 -> f32

    mx = keep.tile([P, NB], F32)
    nc.vector.tensor_reduce(out=mx, in_=lg, axis=AX.X, op=OP.max)

    ex = work.tile([P, NB, C], F32)
    nc.scalar.activation(out=ex, in_=lg, func=AF.Exp)  # |logits| small: no shift
    se = keep.tile([P, NB], F32)
    nc.vector.tensor_reduce(out=se, in_=ex, axis=AX.X, op=OP.add)

    lse = keep.tile([P, NB], F32)
    nc.scalar.activation(out=lse, in_=se, func=AF.Ln)

    emx = work.tile([P, NB], F32)
    nc.scalar.activation(out=emx, in_=mx, func=AF.Exp)
    rse = work.tile([P, NB], F32)
    nc.vector.reciprocal(out=rse, in_=se)
    p_t = keep.tile([P, NB], F32)
    nc.vector.tensor_tensor(out=p_t, in0=emx, in1=rse, op=OP.mult)

    # label logit via one-hot compare + reduce
    eq = work.tile([P, NB, C], F32)
    iota_bc = iota_c[:].rearrange("p (a c) -> p a c", a=1).to_broadcast([P, NB, C])
    labf_bc = labf[:].rearrange("p (n a) -> p n a", a=1).to_broadcast([P, NB, C])
    nc.vector.tensor_tensor(out=eq, in0=iota_bc, in1=labf_bc, op=OP.is_equal)
    lmul = work.tile([P, NB, C], F32)
    nc.vector.tensor_tensor(out=lmul, in0=eq, in1=lg, op=OP.mult)
    ll = keep.tile([P, NB], F32)
    nc.vector.tensor_reduce(out=ll, in_=lmul, axis=AX.X, op=OP.add)

    acc = keep.tile([P, NB], F32)
    nc.vector.tensor_tensor(out=acc, in0=ll, in1=mx, op=OP.is_equal)
    cet = keep.tile([P, NB], F32)
    nc.vector.tensor_tensor(out=cet, in0=lse, in1=ll, op=OP.subtract)

    # ncorrect & ce_sum row-sums; all-reduce across partitions on GpSimd
    # (runs concurrently with the histogram build below)
    stats2 = keep.tile([P, 2], F32)
    nc.vector.tensor_reduce(out=stats2[:, 0:1], in_=acc, axis=AX.X, op=OP.add)
    nc.vector.tensor_reduce(out=stats2[:, 1:2], in_=cet, axis=AX.X, op=OP.add)
    statr = keep.tile([P, 2], F32)
    nc.gpsimd.partition_all_reduce(
        statr, stats2, channels=P, reduce_op=bass_isa.ReduceOp.add
    )
    # rincorrect = (denom != 0) ? 1/denom : 0, with denom = ncorrect - B
    denom = work.tile([P, 1], F32, tag="s1")
    nc.vector.tensor_scalar(
        out=denom, in0=statr[:, 0:1], scalar1=-float(B), scalar2=None, op0=OP.add
    )
    iz = work.tile([P, 1], F32, tag="s2")
    nc.vector.tensor_scalar(
        out=iz, in0=denom, scalar1=0.0, scalar2=None, op0=OP.is_equal
    )
    safe = work.tile([P, 1], F32, tag="s3")
    nc.vector.tensor_tensor(out=safe, in0=denom, in1=iz, op=OP.add)
    rin0 = work.tile([P, 1], F32, tag="s4")
    nc.vector.reciprocal(out=rin0, in_=safe)
    rin_iz = work.tile([P, 1], F32, tag="s5")
    nc.vector.tensor_tensor(out=rin_iz, in0=rin0, in1=iz, op=OP.mult)
    rin = keep.tile([P, 1], F32)
    nc.vector.tensor_tensor(out=rin, in0=rin0, in1=rin_iz, op=OP.subtract)

    # w split: w = w_corr + rin * w_inc (both rin-free)
    #   w_corr = (acc - p) * acc / B ;  w_inc = (acc - p) * (1 - acc)
    amp = work.tile([P, NB], F32)
    nc.vector.tensor_tensor(out=amp, in0=acc, in1=p_t, op=OP.subtract)
    wcr = work.tile([P, NB], F32)
    nc.vector.tensor_tensor(out=wcr, in0=amp, in1=acc, op=OP.mult)
    wpair = keep.tile([P, NB, 2], BF16)
    nc.vector.tensor_scalar(
        out=wpair[:, :, 0], in0=wcr, scalar1=1.0 / B, scalar2=None, op0=OP.mult
    )
    nc.vector.tensor_tensor(out=wpair[:, :, 1], in0=amp, in1=wcr, op=OP.subtract)

    # quantize p -> integer bins (int32 round-trip makes them exact ints)
    qs = work.tile([P, NB], F32)
    nc.vector.tensor_scalar(
        out=qs, in0=p_t, scalar1=QSCALE, scalar2=None, op0=OP.mult
    )
    qi = work.tile([P, NB], I32)
    nc.vector.tensor_copy(out=qi, in_=qs)
    qb = keep.tile([P, NB], BF16)  # bins 0..127: exact in bf16
    nc.vector.tensor_copy(out=qb, in_=qi)

    # one-hot [128, 16, 128] bf16 via one broadcast compare
    oh = keep.tile([P, NB, NBINS], BF16)
    iotab_bc = (
        iota_b[:].rearrange("p (a c) -> p a c", a=1).to_broadcast([P, NB, NBINS])
    )
    qb_bc = qb[:].rearrange("p (n a) -> p n a", a=1).to_broadcast([P, NB, NBINS])
    nc.vector.tensor_tensor(out=oh, in0=qb_bc, in1=iotab_bc, op=OP.is_equal)

    # histogram matmuls with lhsT=oh (m = 128 bins): h lands directly on
    # partitions as PSUM [128, 2] — no PSUM copy / gather DMAs needed
    ps_h = ps_misc.tile([P, 2], F32, tag="misc")
    for n in range(NB):
        nc.tensor.matmul(
            ps_h, oh[:, n, :], wpair[:, n, :],
            start=(n == 0), stop=(n == NB - 1),
        )
    # fold rin: h = h_corr + rin * h_inc (read PSUM directly)
    hio = work.tile([P, 1], F32)
    nc.vector.tensor_scalar(
        out=hio, in0=ps_h[:, 1:2], scalar1=rin[:, 0:1], scalar2=None, op0=OP.mult
    )
    h_t = keep.tile([P, 1], BF16)
    nc.vector.tensor_tensor(out=h_t, in0=ps_h[:, 0:1], in1=hio, op=OP.add)
    h_f = keep.tile([P, 1], F32)
    nc.vector.tensor_tensor(out=h_f, in0=ps_h[:, 0:1], in1=hio, op=OP.add)

    # Th = T @ h (T symmetric: tsb is its own lhsT), then total = h . Th
    ps_th = ps_misc.tile([P, 1], F32, tag="misc")
    nc.tensor.matmul(ps_th, tsb, h_t, start=True, stop=True)
    vw = keep.tile([P, 1], BF16)
    nc.vector.tensor_tensor(out=vw, in0=h_f, in1=ps_th, op=OP.mult)
    ps_f = ps_misc.tile([1, 1], F32, tag="misc")
    nc.tensor.matmul(ps_f, ones_k128, vw, start=True, stop=True)

    lnt = work.tile([1, 1], F32, tag="s6")
    nc.scalar.activation(out=lnt, in_=ps_f, func=AF.Ln)
    outsb = keep.tile([1, 2], F32)
    # mmce = exp(0.5*ln(total) + ln(1/B))  ( = sqrt(total)/B )
    nc.scalar.activation(
        out=outsb[:, 0:1], in_=lnt, func=AF.Exp, bias=lninvb, scale=0.5
    )
    nc.vector.tensor_copy(out=outsb[:, 1:2], in_=statr[0:1, 1:2])
    nc.sync.dma_start(out=out.rearrange("(a b) -> a b", a=1), in_=outsb)

    for pool in reversed(pools):
        pool.release()


def build_nc():
    nc = bacc.Bacc(
        "TRN2",
        target_bir_lowering=False,
        debug=False,
        enable_asserts=False,
        num_devices=N_CORES,
    )
    logits = nc.dram_tensor("logits", [B, C], F32, kind="ExternalInput").ap()
    labels = nc.dram_tensor("labels", [B], I32, kind="ExternalInput").ap()
    out = nc.dram_tensor("out", [2], F32, kind="ExternalOutput").ap()
    t_dram = nc.inline_tensor(_kernel_table(), "ktable").ap()

    with tile.TileContext(nc) as tc:
        _build_body(nc, tc, logits, labels, out, t_dram)
    nc.compile()
    return nc


_NC_CACHE = None


def _get_nc():
    global _NC_CACHE
    if _NC_CACHE is None:
        _NC_CACHE = build_nc()
    return _NC_CACHE


def run(batch_logits, batch_labels, **run_kwargs):
    """Shard, execute on 8 NeuronCores, gather. Returns (loss, results)."""
    nc = _get_nc()
    batch_logits = np.ascontiguousarray(np.asarray(batch_logits, dtype=np.float32))
    labels_i32 = np.ascontiguousarray(np.asarray(batch_labels).astype(np.int32))
    in_maps = [
        {"logits": np.ascontiguousarray(batch_logits[s]), "labels": labels_i32}
        for s in range(N_CORES)
    ]
    res = run_bass_kernel_spmd(nc, in_maps, core_ids=list(range(N_CORES)), **run_kwargs)
    outs = np.stack([np.asarray(r["out"], dtype=np.float64) for r in res.results])
    mmce_mean = outs[:, 0].mean()
    ce = outs[:, 1].sum() / (S * B)
    loss = np.float32(2.0 * mmce_mean + ce)
    return np.asarray(loss, dtype=np.float32), res


def kernel(batch_logits, batch_labels):
    loss, _ = run(batch_logits, batch_labels)
    return loss
